# revision 1
# baseline (speedup 1.0000x reference)
"""Trainium2 Bass kernel for nn_BilinearMLPEmbedPheno.

Pure data-parallel: batch B=131072 sharded over 8 NeuronCores; all weights
replicated. The embedding FFN (relu(e@W1+b1)@W2+b2) is folded into the
gathered table on the host (a weight-only transform); all per-sample compute
runs on device.
"""
import sys

for _p in ("/opt/trn_rl_repo", "/opt/pypackages"):
    if _p not in sys.path:
        sys.path.append(_p)

import numpy as np

import concourse.bass as bass
import concourse.tile as tile
from concourse import bacc, mybir
from concourse.bass_utils import run_bass_kernel_spmd
from concourse.masks import make_identity

F32 = mybir.dt.float32
F32R = mybir.dt.float32r
I16 = mybir.dt.int16

B = 131072
NG = 20000
D = 16
H = 512
NCORES = 8
BL = B // NCORES           # 16384 rows per core
TS = 512                   # samples per tile
T = BL // TS               # 32 tiles
GE = 64                    # gather elem size (fp32) = 256 B


def _build():
    nc = bacc.Bacc("TRN2", target_bir_lowering=False, debug=False)

    # --- DRAM tensors -----------------------------------------------------
    table = nc.dram_tensor("table", [NG, GE], F32, kind="ExternalInput").ap()
    idxs = nc.dram_tensor("idxs", [T, 128, TS * 2 // 16], I16, kind="ExternalInput").ap()
    pht = nc.dram_tensor("pht", [2, BL], F32R, kind="ExternalInput").ap()
    wuv = nc.dram_tensor("wuv", [48, 256], F32R, kind="ExternalInput").ap()
    smat = nc.dram_tensor("smat", [128, 2, 16], F32R, kind="ExternalInput").ap()
    wc2 = nc.dram_tensor("wc2", [16, 1], F32R, kind="ExternalInput").ap()
    wp1 = nc.dram_tensor("wp1", [98, 128], F32R, kind="ExternalInput").ap()
    wp2 = nc.dram_tensor("wp2", [128, 16, 128], F32R, kind="ExternalInput").ap()
    wfin = nc.dram_tensor("wfin", [128, 4], F32R, kind="ExternalInput").ap()
    bc1 = nc.dram_tensor("bc1", [16, 1], F32, kind="ExternalInput").ap()
    fbias = nc.dram_tensor("fbias", [1, 1], F32, kind="ExternalInput").ap()
    out = nc.dram_tensor("out", [T, TS], F32, kind="ExternalOutput").ap()

    GELU = mybir.ActivationFunctionType.Gelu
    RELU = mybir.ActivationFunctionType.Relu

    with tile.TileContext(nc) as tc:
        with tc.tile_pool(name="const", bufs=1) as cp, \
             tc.tile_pool(name="sb", bufs=3) as sb, \
             tc.tile_pool(name="big", bufs=2, space="PSUM") as bigp, \
             tc.tile_pool(name="misc", bufs=4, space="PSUM") as miscp:

            # --- load constants once -------------------------------------
            ident = cp.tile([128, 128], F32)
            make_identity(nc, ident[:])
            wuv_sb = cp.tile([48, 256], F32R)
            nc.sync.dma_start(out=wuv_sb[:], in_=wuv[:])
            s_sb = cp.tile([128, 2, 16], F32R)
            nc.sync.dma_start(out=s_sb[:], in_=smat[:])
            wc2_sb = cp.tile([16, 1], F32R)
            nc.sync.dma_start(out=wc2_sb[:], in_=wc2[:])
            wp1_sb = cp.tile([98, 128], F32R)
            nc.sync.dma_start(out=wp1_sb[:], in_=wp1[:])
            wp2_sb = cp.tile([128, 16, 128], F32R)
            nc.sync.dma_start(out=wp2_sb[:], in_=wp2[:])
            wfin_sb = cp.tile([128, 4], F32R)
            nc.sync.dma_start(out=wfin_sb[:], in_=wfin[:])
            bc1_sb = cp.tile([16, 1], F32)
            nc.sync.dma_start(out=bc1_sb[:], in_=bc1[:])
            fb_sb = cp.tile([1, 1], F32)
            nc.sync.dma_start(out=fb_sb[:], in_=fbias[:])

            for t in range(T):
                cols = slice(t * TS, (t + 1) * TS)

                # --- input DMAs -----------------------------------------
                idx_sb = sb.tile([128, TS * 2 // 16], I16)
                nc.sync.dma_start(out=idx_sb[:], in_=idxs[t])
                ph_sb = sb.tile([98, TS], F32R)
                for m in range(4):
                    nc.sync.dma_start(out=ph_sb[32 * m:32 * m + 2, :], in_=pht[:, cols])

                # --- gather h rows (both slots) -------------------------
                gat_sb = sb.tile([128, 8, GE], F32)
                nc.gpsimd.dma_gather(
                    gat_sb[:], table[:], idx_sb[:], TS * 2, TS * 2, GE,
                )

                # --- pheno branch: g1 = gelu(phenos @ Wp1) --------------
                g1a_ps = bigp.tile([128, 1024], F32, space="PSUM", tag="big")
                g1b_ps = bigp.tile([128, 1024], F32, space="PSUM", tag="big")
                for m in range(4):
                    dst = g1a_ps if m < 2 else g1b_ps
                    nc.tensor.matmul(
                        dst[:, 512 * (m % 2):512 * (m % 2 + 1)],
                        wp1_sb[32 * m:32 * m + 2, :],
                        ph_sb[32 * m:32 * m + 2, :],
                        start=True, stop=True,
                        tile_position=(32 * m, 0),
                    )
                g1t_sb = sb.tile([128, 2048], F32R)
                nc.scalar.activation(g1t_sb[:, 0:1024], g1a_ps[:], GELU)
                nc.scalar.activation(g1t_sb[:, 1024:2048], g1b_ps[:], GELU)

                # --- embed branch: compact (pad 16->32) + 2 transposes --
                cmpa_sb = sb.tile([128, 4, 32], F32)
                cmpb_sb = sb.tile([128, 4, 32], F32)
                nc.gpsimd.tensor_copy(cmpa_sb[:, :, 0:16], gat_sb[:, 0:4, 0:16])
                nc.gpsimd.tensor_copy(cmpb_sb[:, :, 0:16], gat_sb[:, 4:8, 0:16])
                tra_ps = miscp.tile([128, 128], F32, space="PSUM", tag="misc")
                trb_ps = miscp.tile([128, 128], F32, space="PSUM", tag="misc")
                nc.tensor.transpose(tra_ps[:], cmpa_sb[:].rearrange("p g k -> p (g k)"), ident[:])
                nc.tensor.transpose(trb_ps[:], cmpb_sb[:].rearrange("p g k -> p (g k)"), ident[:])
                # psum partitions: 32*c + k
                ht_sb = sb.tile([48, TS], F32R)
                for c in range(4):
                    enga = nc.vector if c < 2 else nc.scalar
                    engb = nc.scalar if c < 2 else nc.vector
                    if c < 2:
                        nc.vector.tensor_copy(ht_sb[0:16, 128 * c:128 * (c + 1)],
                                              tra_ps[32 * c:32 * c + 16, :])
                        nc.scalar.copy(ht_sb[32:48, 128 * c:128 * (c + 1)],
                                       trb_ps[32 * c:32 * c + 16, :])
                    else:
                        nc.scalar.copy(ht_sb[0:16, 128 * c:128 * (c + 1)],
                                       tra_ps[32 * c:32 * c + 16, :])
                        nc.vector.tensor_copy(ht_sb[32:48, 128 * c:128 * (c + 1)],
                                              trb_ps[32 * c:32 * c + 16, :])

                # --- bilinear: U/V matmuls, mul, fused z@Wc1 reduce -----
                w_sb = sb.tile([128, 2, TS], F32R)
                c_ps = miscp.tile([16, TS], F32, space="PSUM", tag="misc")
                for j in range(2):
                    bh = slice(256 * j, 256 * (j + 1))
                    u_ps = miscp.tile([128, 512], F32, space="PSUM", tag="misc")
                    v_ps = miscp.tile([128, 512], F32, space="PSUM", tag="misc")
                    for m in range(2):
                        nc.tensor.matmul(
                            u_ps[:, 256 * m:256 * (m + 1)],
                            wuv_sb[0:16, 128 * m:128 * (m + 1)],
                            ht_sb[0:16, bh],
                            start=True, stop=True,
                        )
                        nc.tensor.matmul(
                            v_ps[:, 256 * m:256 * (m + 1)],
                            wuv_sb[32:48, 128 * m:128 * (m + 1)],
                            ht_sb[32:48, bh],
                            start=True, stop=True,
                        )
                    uc_sb = sb.tile([128, 512], F32)
                    nc.vector.tensor_copy(uc_sb[:], u_ps[:])
                    # w[p, m, 256j:+256] = U * V
                    wdst = bass.AP(
                        tensor=w_sb.tensor,
                        offset=w_sb[:].offset + 256 * j,
                        ap=[w_sb[:].ap[0], [TS, 2], [1, 256]],
                    )
                    nc.vector.tensor_tensor(
                        out=wdst,
                        in0=uc_sb[:].rearrange("p (m b) -> p m b", m=2),
                        in1=v_ps[:].rearrange("p (m b) -> p m b", m=2),
                        op=mybir.AluOpType.mult,
                    )
                    for m in range(2):
                        nc.tensor.matmul(
                            c_ps[:, 256 * j:256 * (j + 1)],
                            s_sb[:, m, :],
                            w_sb[:, m, 256 * j:256 * (j + 1)],
                            start=(m == 0), stop=(m == 1),
                        )

                # --- comb MLP: relu(c + bc1') ---------------------------
                rz_sb = sb.tile([16, TS], F32R)
                nc.vector.tensor_scalar(rz_sb[:], c_ps[:], bc1_sb[:, :], 0.0,
                                        mybir.AluOpType.add, mybir.AluOpType.max)

                # --- Wp2 + gelu2 ----------------------------------------
                g2t_sb = sb.tile([128, 2048], F32R)
                for half in range(2):
                    g2_ps = bigp.tile([128, 1024], F32, space="PSUM", tag="big")
                    for mm in range(2):
                        m = 2 * half + mm
                        for k in range(4):
                            nc.tensor.matmul(
                                g2_ps[:, 512 * mm:512 * (mm + 1)],
                                wp2_sb[:, 4 * k + m, :],
                                g1t_sb[:, 512 * k:512 * (k + 1)],
                                start=(k == 0), stop=(k == 3),
                            )
                    nc.scalar.activation(
                        g2t_sb[:, 1024 * half:1024 * (half + 1)], g2_ps[:], GELU)

                # --- final stacked reduce -------------------------------
                f_ps = miscp.tile([1, TS], F32, space="PSUM", tag="misc")
                for k in range(4):
                    nc.tensor.matmul(
                        f_ps[:], wfin_sb[:, k:k + 1],
                        g2t_sb[:, 512 * k:512 * (k + 1)],
                        start=(k == 0), stop=False,
                    )
                nc.tensor.matmul(f_ps[:], wc2_sb[:], rz_sb[:],
                                 start=False, stop=True)
                out_sb = sb.tile([1, TS], F32)
                nc.vector.tensor_scalar_add(out_sb[:], f_ps[:], fb_sb[:, :])
                nc.sync.dma_start(out=out[t:t + 1, :], in_=out_sb[:])

    nc.compile()
    return nc


_NC_CACHE = None


def _get_nc():
    global _NC_CACHE
    if _NC_CACHE is None:
        _NC_CACHE = _build()
    return _NC_CACHE


def build_in_maps(x, phenos, emb, W1, b1, W2, b2, Wb, ob, Wc1, bc1, Wc2, bc2,
                  Wp1, Wp2, Wp3, bp3):
    x = np.asarray(x)
    phenos = np.asarray(phenos, np.float32)
    emb = np.asarray(emb, np.float32)
    W1 = np.asarray(W1, np.float32); b1 = np.asarray(b1, np.float32)
    W2 = np.asarray(W2, np.float32); b2 = np.asarray(b2, np.float32)
    Wb = np.asarray(Wb, np.float32); ob = np.asarray(ob, np.float32)
    Wc1 = np.asarray(Wc1, np.float32); bc1 = np.asarray(bc1, np.float32)
    Wc2 = np.asarray(Wc2, np.float32); bc2 = np.asarray(bc2, np.float32)
    Wp1 = np.asarray(Wp1, np.float32); Wp2 = np.asarray(Wp2, np.float32)
    Wp3 = np.asarray(Wp3, np.float32); bp3 = np.asarray(bp3, np.float32)

    # --- weight-only precompute: fold embedding_ffn into the table --------
    h = np.maximum(emb.astype(np.float64) @ W1.astype(np.float64) + b1, 0.0)
    h = h @ W2.astype(np.float64) + b2                     # [NG, D]
    table = np.zeros((NG, GE), np.float32)
    table[:, :D] = h.astype(np.float32)

    # U[b,(a2,a1)] = sum_k Wb[a2,a1,k] h0[b,k]; V[b,(a2,a1)] = sum_k Wb[a1,a2,k] h1[b,k]
    wuv = np.zeros((48, 256), np.float32)
    wuv[0:16] = Wb.transpose(2, 0, 1).reshape(16, 256)
    wuv[32:48] = Wb.transpose(2, 1, 0).reshape(16, 256)

    # fused z@Wc1: SC[(a2,a1), o] = Wc1[a2, o] -> c = SC.T @ W directly
    sc = np.repeat(Wc1, 16, axis=0).astype(np.float32)          # [256, 16]
    smat = sc.reshape(2, 128, 16).transpose(1, 0, 2).copy()     # [128, 2, 16]
    bc1f = (bc1.reshape(16) + ob.reshape(16) @ Wc1).astype(np.float32)

    wp1 = np.zeros((98, 128), np.float32)
    for m in range(4):
        wp1[32 * m:32 * m + 2, :] = Wp1[:, 128 * m:128 * (m + 1)]
    wp2 = np.zeros((128, 16, 128), np.float32)
    for k in range(4):
        for m in range(4):
            wp2[:, 4 * k + m, :] = Wp2[128 * k:128 * (k + 1), 128 * m:128 * (m + 1)]
    wfin = Wp3.reshape(4, 128).T.copy()                    # [128, 4]

    fbias = np.float32(bc2.reshape(()) + bp3.reshape(()))

    # --- per-core sharding + index marshalling ---------------------------
    in_maps = []
    xi = x.astype(np.int64)
    for core in range(NCORES):
        xl = xi[core * BL:(core + 1) * BL]                 # [BL, 2]
        phl = phenos[core * BL:(core + 1) * BL]            # [BL, 2]
        # gather order i = g*128 + b, g = s*4 + c; sample row = t*512 + c*128 + b
        i = np.arange(TS * 2)
        g, bb = i // 128, i % 128
        s, c = g // 4, g % 4
        rows = (np.arange(T)[:, None] * TS + c[None, :] * 128 + bb[None, :])  # [T, 1024]
        vals = xl[rows, s[None, :]].astype(np.int16)       # [T, 1024]
        wrap = np.zeros((T, 16, TS * 2 // 16), np.int16)
        wrap[:, i % 16, i // 16] = vals
        idx_all = np.tile(wrap, (1, 8, 1))                 # [T, 128, 64]

        in_maps.append(dict(
            table=table,
            idxs=idx_all,
            pht=np.ascontiguousarray(phl.T),
            wuv=wuv,
            smat=smat,
            wc2=Wc2,
            wp1=wp1, wp2=wp2, wfin=wfin,
            bc1=bc1f.reshape(16, 1),
            fbias=fbias.reshape(1, 1),
        ))

    return in_maps


def kernel(**inputs):
    in_maps = build_in_maps(**inputs)
    nc = _get_nc()
    res = run_bass_kernel_spmd(nc, in_maps, core_ids=list(range(NCORES)))
    return np.concatenate([res.results[i]["out"].reshape(BL) for i in range(NCORES)]).astype(np.float32)

